# revision 5
# baseline (speedup 1.0000x reference)
"""CopyGenerator kernel for 8 Trainium2 NeuronCores.

Strategy:
  - Tensor-parallel shard the 32k-vocab output projection across the 8
    cores (4000 rows each) with a cross-core AllReduce for the softmax
    normalizer; data-parallel shard the ext-vocab scatter over batch
    (4 batches per core).
  - The big projection runs in fp16 on the PE (fp32 PSUM accumulate).
  - Scatter-add is a onehot matmul: onehot[s,e] = (idx[s] == e), built
    with iota + is_equal on the vector engine.
Host-side work is limited to layout marshalling (transpose/shard/cast of
inputs, concatenation of outputs).
"""
import sys
sys.path.insert(0, "/opt/trn_rl_repo")
import numpy as np

TLEN, BSZ, HID = 64, 32, 1024
SLEN, V_TGT, V_EXT = 200, 32000, 2000
NCORES = 8
VSH = V_TGT // NCORES          # 4000 vocab rows per core
BSH = BSZ // NCORES            # 4 batches per core (ext scatter)
NROWS = TLEN * BSZ             # 2048
NT = NROWS // 128              # 16 row tiles
KB = HID // 128                # 8 contraction chunks
VC = 500                       # vocab free-dim chunk (1 PSUM bank)
NVC = VSH // VC                # 8
GRP = 2                        # row tiles per softmax AllReduce group
NG = NT // GRP
SA, SB_ = 128, SLEN - 128      # source-len split (128 + 72)
EC = 500                       # ext chunk
NEC = V_EXT // EC              # 4
LOG_LO = float(np.log(0.001))

_prog_cache = {}


def _build_program(has_bout: bool, neg_bcopy: float):
    import concourse.bacc as bacc
    import concourse.tile as tile
    import concourse.mybir as mybir

    f32, f16, i32 = mybir.dt.float32, mybir.dt.float16, mybir.dt.int32
    AF = mybir.ActivationFunctionType
    OP = mybir.AluOpType

    nc = bacc.Bacc("TRN2", target_bir_lowering=False, debug=False,
                   num_devices=NCORES)

    hT = nc.dram_tensor("hT", [HID, NROWS], f32, kind="ExternalInput")
    WT = nc.dram_tensor("WT", [HID, VSH], f32, kind="ExternalInput")
    wck = nc.dram_tensor("wck", [128, KB], f32, kind="ExternalInput")
    attnT = nc.dram_tensor("attnT", [BSH, SLEN, TLEN], f32, kind="ExternalInput")
    idxc = nc.dram_tensor("idxc", [BSH, SLEN], i32, kind="ExternalInput")
    hxT = nc.dram_tensor("hxT", [BSH, HID, TLEN], f32, kind="ExternalInput")
    if has_bout:
        bb = nc.dram_tensor("bb", [128, VSH], f32, kind="ExternalInput")
    vout = nc.dram_tensor("vout", [NROWS, VSH], f32, kind="ExternalOutput")
    eout = nc.dram_tensor("eout", [TLEN, BSH, V_EXT], f32, kind="ExternalOutput")

    with tile.TileContext(nc) as tc:
        with (
            tc.tile_pool(name="wt", bufs=1) as wt_pool,
            tc.tile_pool(name="const", bufs=1) as const_pool,
            tc.tile_pool(name="ht", bufs=3) as ht_pool,
            tc.tile_pool(name="lt", bufs=2 * GRP + 2) as lt_pool,
            tc.tile_pool(name="esc", bufs=4) as esc_pool,
            tc.tile_pool(name="sep", bufs=3) as sep_pool,
            tc.tile_pool(name="stage", bufs=6) as stage_pool,
            tc.tile_pool(name="small", bufs=4) as small_pool,
            tc.tile_pool(name="cc", bufs=2 * NG) as cc_pool,
            tc.tile_pool(name="ext", bufs=2) as ext_pool,
            tc.tile_pool(name="psmm", bufs=4, space="PSUM") as psmm,
            tc.tile_pool(name="psz", bufs=2, space="PSUM") as psz,
            tc.tile_pool(name="pse", bufs=2, space="PSUM") as pse,
            tc.tile_pool(name="dram", bufs=2 * NG + 2, space="DRAM") as dram_pool,
        ):
            # ---- persistent tiles -------------------------------------
            wt_sb = wt_pool.tile([128, KB, VSH], f16)
            for vc in range(NVC):
                src = WT[:, vc * VC:(vc + 1) * VC].rearrange(
                    "(kb p) v -> p kb v", p=128)
                nc.gpsimd.dma_start(wt_sb[:, :, vc * VC:(vc + 1) * VC], src)

            wck_sb = const_pool.tile([128, KB], f16)
            nc.gpsimd.dma_start(wck_sb[:], wck[:, :])

            lcs_all = const_pool.tile([128, NT], f32)   # ln(clip(sigmoid(z)))

            # ---- main vocab loop --------------------------------------
            lt_tiles = {}
            cc_sb = {}
            for g in range(NG):
                cc_sb[g] = cc_pool.tile([128, GRP], f32, tag="ccin", name=f"ccin{g}")

            for tt in range(NT):
                g, j = divmod(tt, GRP)
                ht_sb = ht_pool.tile([128, KB, 128], f16)
                src = hT[:, tt * 128:(tt + 1) * 128].rearrange(
                    "(kb p) t -> p kb t", p=128)
                nc.gpsimd.dma_start(ht_sb[:], src)

                # copy gate z = h . w_copy  (accumulate over KB chunks)
                zp = psz.tile([128, 1], f32, tag="zp")
                for kb in range(KB):
                    nc.tensor.matmul(zp[:], ht_sb[:, kb, :], wck_sb[:, kb:kb + 1],
                                     start=(kb == 0), stop=(kb == KB - 1))
                # sigmoid via exp/reciprocal (stay in the exp/ln table set)
                e_t = small_pool.tile([128, 1], f32, tag="e")
                nc.scalar.activation(e_t[:], zp[:], AF.Exp,
                                     scale=-1.0, bias=neg_bcopy)
                sp = small_pool.tile([128, 1], f32, tag="sp")
                nc.vector.tensor_scalar_add(sp[:], e_t[:], 1.0)
                sig = small_pool.tile([128, 1], f32, tag="sig")
                nc.vector.reciprocal(sig[:], sp[:])     # sigmoid(z + b_copy)
                cl = small_pool.tile([128, 1], f32, tag="cl")
                nc.vector.tensor_scalar(cl[:], sig[:], 0.001, 0.999,
                                        op0=OP.max, op1=OP.min)
                nc.scalar.activation(lcs_all[:, tt:tt + 1], cl[:], AF.Ln)

                # logits for this row tile, all vocab chunks
                lt16 = lt_pool.tile([128, VSH], f16, tag="lt")
                lt_tiles[tt] = lt16
                sep = sep_pool.tile([128, NVC], f32, tag="sep")
                for vc in range(NVC):
                    sl = slice(vc * VC, (vc + 1) * VC)
                    pm = psmm.tile([128, VC], f32, tag="pm")
                    for kb in range(KB):
                        nc.tensor.matmul(pm[:], ht_sb[:, kb, :], wt_sb[:, kb, sl],
                                         start=(kb == 0), stop=(kb == KB - 1))
                    if has_bout:
                        nc.vector.tensor_add(pm[:], pm[:], bb[:, sl])
                    nc.scalar.activation(lt16[:, sl], pm[:], AF.Copy)
                    esc = esc_pool.tile([128, VC], f16, tag="esc")
                    nc.scalar.activation(esc[:], pm[:], AF.Exp,
                                         accum_out=sep[:, vc:vc + 1])
                nc.vector.tensor_reduce(cc_sb[g][:, j:j + 1], sep[:],
                                        axis=mybir.AxisListType.X, op=OP.add)

                if j == GRP - 1:
                    # softmax-sum AllReduce for this group of row tiles
                    cin = dram_pool.tile([128, GRP], f32, tag="cin")
                    cout = dram_pool.tile([128, GRP], f32, tag="cout")
                    nc.sync.dma_start(cin[:], cc_sb[g][:])
                    nc.gpsimd.collective_compute(
                        "AllReduce", OP.add,
                        replica_groups=[list(range(NCORES))],
                        ins=[cin[:]], outs=[cout[:]],
                    )
                    s_sb = small_pool.tile([128, GRP], f32, tag="ssb")
                    nc.sync.dma_start(s_sb[:], cout[:])
                    lns = small_pool.tile([128, GRP], f32, tag="lns")
                    nc.scalar.activation(lns[:], s_sb[:], AF.Ln)
                    negc = small_pool.tile([128, GRP], f32, tag="negc")
                    nc.vector.tensor_sub(
                        negc[:], lcs_all[:, g * GRP:(g + 1) * GRP], lns[:])
                    for jj in range(GRP):
                        t2 = g * GRP + jj
                        lt2 = lt_tiles.pop(t2)
                        for vc in range(NVC):
                            sl = slice(vc * VC, (vc + 1) * VC)
                            st = stage_pool.tile([128, VC], f32, tag="st")
                            nc.vector.tensor_scalar_add(st[:], lt2[:, sl],
                                                        negc[:, jj:jj + 1])
                            nc.sync.dma_start(
                                vout[t2 * 128:(t2 + 1) * 128, sl], st[:])

            # ---- ext-vocab scatter (batch-sharded) --------------------
            iota_i = const_pool.tile([128, V_EXT], i32)
            nc.gpsimd.iota(iota_i[:], pattern=[[1, V_EXT]], base=0,
                           channel_multiplier=0)
            iota_sb = const_pool.tile([128, V_EXT], f32)
            nc.vector.tensor_copy(iota_sb[:], iota_i[:])
            for b in range(BSH):
                # z for this batch's (t, b) rows -> sigma = 1 - sigmoid
                hx_sb = ext_pool.tile([128, KB, TLEN], f16, tag="hx")
                nc.gpsimd.dma_start(
                    hx_sb[:], hxT[b].rearrange("(kb p) t -> p kb t", p=128))
                zx = psz.tile([128, 1], f32, tag="zp")
                for kb in range(KB):
                    nc.tensor.matmul(zx[:TLEN], hx_sb[:, kb, :],
                                     wck_sb[:, kb:kb + 1],
                                     start=(kb == 0), stop=(kb == KB - 1))
                ex = small_pool.tile([TLEN, 1], f32, tag="ex")
                nc.scalar.activation(ex[:], zx[:TLEN], AF.Exp,
                                     scale=-1.0, bias=neg_bcopy)
                spx = small_pool.tile([TLEN, 1], f32, tag="spx")
                nc.vector.tensor_scalar_add(spx[:], ex[:], 1.0)
                ivx = small_pool.tile([TLEN, 1], f32, tag="ivx")
                nc.vector.reciprocal(ivx[:], spx[:])
                sgx = small_pool.tile([TLEN, 1], f32, tag="sgx")
                nc.vector.tensor_mul(sgx[:], ex[:], ivx[:])   # 1 - sigmoid

                # onehot tiles from indices
                idx_i = ext_pool.tile([128, 2], i32, tag="idxi")
                nc.sync.dma_start(idx_i[:SA, 0:1],
                                  idxc[b:b + 1, 0:SA].rearrange("o s -> s o"))
                nc.sync.dma_start(idx_i[:SB_, 1:2],
                                  idxc[b:b + 1, SA:SLEN].rearrange("o s -> s o"))
                idx_sb = ext_pool.tile([128, 2], f32, tag="idx")
                nc.vector.tensor_copy(idx_sb[:SA, 0:1], idx_i[:SA, 0:1])
                nc.vector.tensor_copy(idx_sb[:SB_, 1:2], idx_i[:SB_, 1:2])
                oh_a = ext_pool.tile([128, V_EXT], f16, tag="oha")
                oh_b = ext_pool.tile([128, V_EXT], f16, tag="ohb")
                nc.vector.tensor_scalar(oh_a[:], iota_sb[:], idx_sb[:, 0:1],
                                        None, op0=OP.is_equal)
                nc.vector.tensor_scalar(oh_b[:SB_], iota_sb[:SB_],
                                        idx_sb[:SB_, 1:2], None,
                                        op0=OP.is_equal)

                # attn^T tiles (s on partitions), fp16
                at_a = ext_pool.tile([128, TLEN], f16, tag="ata")
                at_b = ext_pool.tile([128, TLEN], f16, tag="atb")
                nc.gpsimd.dma_start(at_a[:], attnT[b, 0:SA, :])
                nc.gpsimd.dma_start(at_b[:SB_], attnT[b, SA:SLEN, :])

                for ec in range(NEC):
                    sl = slice(ec * EC, (ec + 1) * EC)
                    pe_ = pse.tile([TLEN, EC], f32, tag="pe")
                    nc.tensor.matmul(pe_[:], at_a[:], oh_a[:, sl],
                                     start=True, stop=False)
                    nc.tensor.matmul(pe_[:], at_b[:SB_], oh_b[:SB_, sl],
                                     start=False, stop=True)
                    est = stage_pool.tile([TLEN, EC], f32, tag="est")
                    nc.vector.tensor_scalar(est[:], pe_[:], sgx[:], 0.001,
                                            op0=OP.mult, op1=OP.max)
                    nc.vector.tensor_scalar_min(est[:], est[:], 0.999)
                    elg = stage_pool.tile([TLEN, EC], f32, tag="elg")
                    nc.scalar.activation(elg[:], est[:], AF.Ln)
                    if ec == 0:
                        nc.vector.memset(elg[:, 0:1], LOG_LO)
                    nc.sync.dma_start(eout[:, b, sl], elg[:])

    nc.compile()
    return nc


def _get_program(has_bout: bool, neg_bcopy: float):
    key = (has_bout, neg_bcopy)
    if key not in _prog_cache:
        _prog_cache[key] = _build_program(has_bout, neg_bcopy)
    return _prog_cache[key]


def kernel(hidden, attn, copy_to_ext, W_out, b_out, w_copy, b_copy):
    from concourse.bass_utils import run_bass_kernel_spmd

    h2 = np.asarray(hidden, np.float32).reshape(NROWS, HID)
    hT_host = np.ascontiguousarray(h2.T)                       # [1024, 2048]
    a2 = np.asarray(attn, np.float32)                          # [64, 32, 200]
    attnT_full = np.ascontiguousarray(a2.transpose(1, 2, 0))   # [32, 200, 64]
    idx_full = np.ascontiguousarray(
        np.asarray(copy_to_ext).astype(np.int32).T)            # [32, 200]
    W = np.asarray(W_out, np.float32)
    bo = np.asarray(b_out, np.float32)
    wc = np.asarray(w_copy, np.float32).reshape(HID)
    wck_host = np.ascontiguousarray(wc.reshape(KB, 128).T)     # [128, KB]
    neg_bcopy = -float(np.asarray(b_copy, np.float32).reshape(-1)[0])
    has_bout = bool(np.any(bo))

    nc = _get_program(has_bout, neg_bcopy)

    in_maps = []
    for c in range(NCORES):
        WTc = np.ascontiguousarray(W[c * VSH:(c + 1) * VSH].T)  # [1024, 4000]
        bsl = slice(c * BSH, (c + 1) * BSH)
        hx = np.stack([np.ascontiguousarray(h2[(c * BSH + b)::BSZ, :].T)
                       for b in range(BSH)])                    # [4, 1024, 64]
        m = {
            "hT": hT_host,
            "WT": WTc,
            "wck": wck_host,
            "attnT": np.ascontiguousarray(attnT_full[bsl]),
            "idxc": np.ascontiguousarray(idx_full[bsl]),
            "hxT": hx,
        }
        if has_bout:
            m["bb"] = np.ascontiguousarray(
                np.broadcast_to(bo[c * VSH:(c + 1) * VSH], (128, VSH)))
        in_maps.append(m)

    res = run_bass_kernel_spmd(nc, in_maps, core_ids=list(range(NCORES)))

    out = np.empty((NROWS, V_TGT + V_EXT), np.float32)
    out3 = out.reshape(TLEN, BSZ, V_TGT + V_EXT)
    for c in range(NCORES):
        out[:, c * VSH:(c + 1) * VSH] = res.results[c]["vout"]
        out3[:, c * BSH:(c + 1) * BSH, V_TGT:] = res.results[c]["eout"]
    return out3
